# revision 20
# baseline (speedup 1.0000x reference)
"""LGRU Bass/Tile kernel for Trainium2, 8-core data-parallel over batch.

Reference computation (per sequence step t):
    xz = x @ Wz ; xh = x @ Wh                     (input projections)
    z  = sigmoid(xz_t + h @ Uz)
    hc = relu(xh_t + h @ Uh)
    h  = z * h + (1 - z) * hc
Returns all hidden states hs[T, B, H].

Sharding: batch (B=32) split 4-per-core across 8 cores; weights replicated.

Kernel design (v3):
  - h lives TRANSPOSED in SBUF as hsT[128, kc, t'*BL+b] (H on partitions)
    in bf16; the state buffer doubles as the block's output staging.
  - Per-step matmuls use U chunks as stationary bf16 operands with the
    tiny h slice moving, accumulating into per-step PSUM tiles that are
    PRE-FILLED with the x-projections via a bf16 identity matmul. Every
    per-step PSUM tile occupies a full 2 KiB bank (no false bank-sharing
    between the PE and the DVE/ACT readers).
  - The sigmoid runs ON THE VECTOR ENGINE via the Schraudolph exp bit
    trick (ScalarE's issue->semaphore-visible latency of ~1.1us would
    otherwise sit on the loop-carried path):
        m = A*az + B  (converted to int32; A=2^23/ln2, B=127*2^23-C)
        e = max(bitcast_f32(m), 0)    (clamp handles az < -8.8 garbage)
        w = 1/(1+e)  via reciprocal_approx_fast (51 ULP)  = 1 - z
        f = relu(ah) - h_prev         (fused scalar_tensor_tensor)
        h = h_prev + w * f
    f is written into d's buffer: the WAR hazard pins f behind the
    sigmoid chain in the DVE queue (the scheduler's cost model under-
    estimates the matmul bursts and would otherwise stall the DVE).
    End-to-end rel-L2 vs the fp32 reference: 6.4e-3 (gate is 2e-2).
  - Software pipelining: the loop body covers TWO blocks with ping-pong
    x-tile sets; block j+1's x DMA/transposes/projections are emitted
    interleaved into block j's recurrence steps. The projection PSUM
    tiles come from the SAME rotating PSUM ring as the per-step tiles,
    so ring reuse (write-after-read) pins each projection matmul near
    its emission slot — without that, the ASAP scheduler would front-
    load all prep at the block boundary and recreate the serial bubble.
    All prefetch copies (PSUM evacuation, casts) run on the otherwise
    idle Scalar engine. x is padded by one block host-side so the last
    prefetch never reads out of bounds.
  - Output is DMA'd in the transposed layout and un-transposed on the
    host (a PE-transpose + copy per block was pure overhead).
"""

import os

import numpy as np

T, B, F, H = 2048, 32, 256, 512
NCORES = 8
BL = B // NCORES  # batch per core = 4
TBLK = 128  # timesteps per block
KC = H // 128  # 4 H-chunks
FC = F // 128  # 2 F-chunks
PT = (TBLK * BL) // 128  # 4 partition-tiles of (t,b) rows per block (== KC)

SIG_A = float(2**23 / np.log(2))
SIG_B = float(127 * 2**23 - 500000)

_CACHED = {}


def _build_nc(t_total):
    import concourse.mybir as mybir
    from concourse import bacc
    import concourse.tile as tile
    from concourse.bass import ds
    from concourse.masks import make_identity

    FD = mybir.dt.float32
    BF = mybir.dt.bfloat16
    I32 = mybir.dt.int32
    nblk = t_total // TBLK
    paired = nblk % 2 == 0 and nblk >= 2

    nc = bacc.Bacc("TRN2", target_bir_lowering=False, debug=False)
    # one pad block so next-block prefetch never reads out of bounds
    x = nc.dram_tensor("x", [t_total + TBLK, BL, F], FD, kind="ExternalInput")
    Wz = nc.dram_tensor("Wz", [F, H], FD, kind="ExternalInput")
    Wh = nc.dram_tensor("Wh", [F, H], FD, kind="ExternalInput")
    Uz = nc.dram_tensor("Uz", [H, H], FD, kind="ExternalInput")
    Uh = nc.dram_tensor("Uh", [H, H], FD, kind="ExternalInput")
    # transposed output: hs[blk, p, c, tp*BL+b] = h[blk*TBLK+tp, b, c*128+p]
    hs = nc.dram_tensor(
        "hs", [nblk, 128, KC, TBLK * BL], FD, kind="ExternalOutput"
    )

    x_flat = x.rearrange("t b f -> (t b) f")
    hs_flat = hs.rearrange("a p c t -> (a p) c t")

    Alu = mybir.AluOpType

    with tile.TileContext(nc) as tc:
        with (
            tc.tile_pool(name="const", bufs=1) as constp,
            tc.tile_pool(name="setup", bufs=2) as setupp,
            tc.tile_pool(name="state", bufs=1) as statep,
            tc.tile_pool(name="xblk", bufs=1) as xblkp,
            tc.tile_pool(name="work", bufs=3) as workp,
            tc.tile_pool(name="step", bufs=3) as stepp,
            tc.tile_pool(name="ps_rec", bufs=6, space="PSUM") as ps_rec,
            tc.tile_pool(name="ps_tr", bufs=2, space="PSUM") as ps_tr,
        ):
            ident = constp.tile([128, 128], FD, tag="ident")
            make_identity(nc, ident)
            ident_b = constp.tile([128, 128], BF, tag="identb")
            nc.vector.tensor_copy(ident_b, ident)
            ones = constp.tile([128, KC, BL], FD, tag="ones")
            nc.vector.memset(ones, 1.0)

            # --- U blocks, single bf16 ---
            Ub = {}
            for g, Usrc in (("z", Uz), ("h", Uh)):
                for kc in range(KC):
                    stage = setupp.tile(
                        [128, H], FD, tag=f"stage{g}{kc}", name=f"stage{g}{kc}"
                    )
                    nc.sync.dma_start(out=stage, in_=Usrc[kc * 128 : (kc + 1) * 128, :])
                    ub = constp.tile([128, H], BF, tag=f"U{g}{kc}")
                    nc.vector.tensor_copy(ub, stage)
                    Ub[(g, kc)] = ub

            # --- W blocks, bf16: Wcat = [Wz | Wh] along output dim ---
            Wb = []
            for kc in range(FC):
                wtile = constp.tile([128, 2 * H], BF, tag=f"W{kc}")
                for si, Wsrc in enumerate((Wz, Wh)):
                    stage = setupp.tile(
                        [128, H], FD, tag=f"stageW{kc}{si}", name=f"stageW{kc}{si}"
                    )
                    nc.sync.dma_start(out=stage, in_=Wsrc[kc * 128 : (kc + 1) * 128, :])
                    nc.vector.tensor_copy(wtile[:, si * H : (si + 1) * H], stage)
                Wb.append(wtile)

            # --- persistent state: transposed h states for one block, bf16 ---
            hsT = statep.tile([128, KC, TBLK * BL], BF)
            nc.vector.memset(hsT[:, :, (TBLK - 1) * BL :], 0.0)

            # --- ping-pong x-tile sets ---
            def make_set(sfx):
                return {
                    "xT": [
                        xblkp.tile(
                            [128, KC, 128], BF, tag=f"xT{fc}{sfx}", name=f"xT{fc}{sfx}"
                        )
                        for fc in range(FC)
                    ],
                    "xzT": xblkp.tile(
                        [128, KC, KC, 128], BF, tag=f"xzT{sfx}", name=f"xzT{sfx}"
                    ),
                    "xhT": xblkp.tile(
                        [128, KC, KC, 128], BF, tag=f"xhT{sfx}", name=f"xhT{sfx}"
                    ),
                }

            set_a = make_set("a")
            set_b = make_set("b")

            def prep_items(dst, row0):
                """Closures loading/transforming block data at rows row0 into
                tile set dst. PE work rides the rec-PSUM ring (pinned); all
                copies run on the Scalar engine."""
                items = []
                shared = {}

                def mk_load(pt):
                    def it():
                        xin = workp.tile([128, F], FD, tag="xin", bufs=4, name="xin")
                        nc.sync.dma_start(
                            out=xin, in_=x_flat[ds(row0 + pt * 128, 128), :]
                        )
                        xb = workp.tile([128, F], BF, tag="xb", bufs=4, name="xb")
                        nc.scalar.copy(xb, xin)
                        shared[pt] = xb

                    return it

                def mk_tr(pt, fc):
                    def it():
                        pst = ps_tr.tile([128, 128], BF, tag="trb", name="pst")
                        nc.tensor.transpose(
                            pst, shared[pt][:, fc * 128 : (fc + 1) * 128], ident_b
                        )
                        nc.scalar.copy(dst["xT"][fc][:, pt, :], pst)

                    return it

                def mk_proj(mt):
                    def it():
                        psp = ps_rec.tile(
                            [128, KC, 128], FD, tag="rec", name="psp"
                        )
                        lhs_sl = slice(mt * 128, (mt + 1) * 128)
                        for kc in range(FC):
                            nc.tensor.matmul(
                                psp,
                                lhsT=Wb[kc][:, lhs_sl],
                                rhs=dst["xT"][kc],
                                start=(kc == 0),
                                stop=(kc == FC - 1),
                            )
                        d = dst["xzT"] if mt < KC else dst["xhT"]
                        nc.scalar.copy(d[:, mt % KC, :, :], psp)

                    return it

                for pt in range(PT):
                    items.append(mk_load(pt))
                    for fc in range(FC):
                        items.append(mk_tr(pt, fc))
                for mt in range(2 * KC):
                    items.append(mk_proj(mt))
                return items

            def emit_step(cur, tp, extra):
                q, r = divmod(tp * BL, 128)
                curs = ds(tp * BL, BL)
                prev = ds((tp - 1) * BL, BL) if tp > 0 else ds((TBLK - 1) * BL, BL)
                ps_z = ps_rec.tile([128, KC, 128], FD, tag="rec", name="ps_z")
                ps_h = ps_rec.tile([128, KC, 128], FD, tag="rec", name="ps_h")
                # prefill PSUM with the x-projections (identity matmul sets
                # has_written so the U matmuls accumulate on top)
                nc.tensor.matmul(
                    ps_z[:, :, 0:BL],
                    lhsT=ident_b,
                    rhs=cur["xzT"][:, :, q, r : r + BL],
                    start=True,
                    stop=False,
                )
                nc.tensor.matmul(
                    ps_h[:, :, 0:BL],
                    lhsT=ident_b,
                    rhs=cur["xhT"][:, :, q, r : r + BL],
                    start=True,
                    stop=False,
                )

                def u_burst(g, ps, kc_outer):
                    # kc_outer: the first matmuls need only the first h
                    # chunks, so the burst can start before the split
                    # h-write (below) fully completes
                    loops = (
                        [(mt, kc) for kc in range(KC) for mt in range(KC)]
                        if kc_outer
                        else [(mt, kc) for mt in range(KC) for kc in range(KC)]
                    )
                    for mt, kc in loops:
                        nc.tensor.matmul(
                            ps[:, mt, 0:BL],
                            lhsT=Ub[(g, kc)][:, mt * 128 : (mt + 1) * 128],
                            rhs=hsT[:, kc, prev],
                            start=False,
                            stop=(kc == KC - 1),
                            skip_group_check=True,
                        )

                u_burst("z", ps_z, kc_outer=True)
                # w = 1-z = 1/(1+exp(az)) via DVE exp bit trick, overlapping
                # the h matmul burst
                u_t = stepp.tile([128, KC, BL], I32, tag="u", name="u_t")
                nc.vector.tensor_scalar(
                    u_t, ps_z[:, :, 0:BL], SIG_A, SIG_B, Alu.mult, Alu.add
                )
                d_t = stepp.tile([128, KC, BL], FD, tag="d", name="d_t")
                nc.vector.tensor_scalar(
                    d_t, u_t.bitcast(FD), 0.0, 1.0, Alu.max, Alu.add
                )
                w_t = stepp.tile([128, KC, BL], FD, tag="w", name="w_t")
                # d is in [1, ~6600] — safely inside approx_fast's domain
                nc.vector.reciprocal_approx_fast(out=w_t, in_=d_t)
                u_burst("h", ps_h, kc_outer=False)
                # f = relu(ah) - h_prev, fused; written INTO d_t: the WAR
                # hazard (w reads d_t) pins f behind the sigmoid chain in
                # the DVE queue at zero cost
                f_t = d_t
                nc.vector.scalar_tensor_tensor(
                    f_t, ps_h[:, :, 0:BL], 0.0, hsT[:, :, prev], Alu.max, Alu.subtract
                )
                # h = h_prev + w*f, written straight into bf16 state, in two
                # halves so the next kc-outer z-burst can start early
                g_t = stepp.tile([128, KC, BL], FD, tag="g", name="g_t")
                nc.vector.tensor_mul(g_t, w_t, f_t)
                half = KC // 2
                nc.vector.tensor_add(
                    hsT[:, 0:half, curs], g_t[:, 0:half, :], hsT[:, 0:half, prev]
                )
                nc.vector.tensor_add(
                    hsT[:, half:, curs], g_t[:, half:, :], hsT[:, half:, prev]
                )
                if extra is not None:
                    extra(tp)

            def emit_block(cur, hsrow0, prefetch_items):
                if prefetch_items:
                    sched = {}
                    for k, it in enumerate(prefetch_items):
                        sched[2 + 3 * k] = it

                    def extra(tp):
                        if tp in sched:
                            sched[tp]()

                else:
                    extra = None
                for tp in range(TBLK):
                    emit_step(cur, tp, extra)
                # cast block states to f32 on the Scalar engine and DMA out
                hsF = workp.tile(
                    [128, KC, TBLK * BL], FD, tag="hsF", bufs=2, name="hsF"
                )
                for c in range(KC):
                    nc.scalar.copy(hsF[:, c, :], hsT[:, c, :])
                nc.sync.dma_start(out=hs_flat[ds(hsrow0, 128), :, :], in_=hsF)

            # prologue: block 0 into set A
            for it in prep_items(set_a, 0):
                it()

            if paired:
                with tc.For_i(0, nblk // 2, 1, staggered_reset=True) as pair:
                    row_even = pair * (2 * TBLK * BL)
                    emit_block(
                        set_a,
                        pair * 256,
                        prep_items(set_b, row_even + TBLK * BL),
                    )
                    emit_block(
                        set_b,
                        pair * 256 + 128,
                        prep_items(set_a, row_even + 2 * TBLK * BL),
                    )
            else:
                with tc.For_i(0, nblk, 1, staggered_reset=True) as blk:
                    emit_block(
                        set_a,
                        blk * 128,
                        prep_items(set_a, blk * (TBLK * BL) + TBLK * BL)
                        if nblk > 1
                        else None,
                    )

    nc.finalize()
    return nc


def kernel(x, Wz, Wh, Uz, Uh):
    from concourse.bass_utils import run_bass_kernel_spmd

    t_total = x.shape[0]
    if t_total not in _CACHED:
        _CACHED[t_total] = _build_nc(t_total)
    nc = _CACHED[t_total]

    x = np.asarray(x, dtype=np.float32)
    xpad = np.zeros((t_total + TBLK, x.shape[1], x.shape[2]), dtype=np.float32)
    xpad[:t_total] = x
    Wz = np.ascontiguousarray(np.asarray(Wz, dtype=np.float32))
    Wh = np.ascontiguousarray(np.asarray(Wh, dtype=np.float32))
    Uz = np.ascontiguousarray(np.asarray(Uz, dtype=np.float32))
    Uh = np.ascontiguousarray(np.asarray(Uh, dtype=np.float32))

    in_maps = []
    for c in range(NCORES):
        in_maps.append(
            {
                "x": np.ascontiguousarray(xpad[:, c * BL : (c + 1) * BL, :]),
                "Wz": Wz,
                "Wh": Wh,
                "Uz": Uz,
                "Uh": Uh,
            }
        )

    trace = os.environ.get("LGRU_TRACE", "0") == "1"
    res = run_bass_kernel_spmd(
        nc, in_maps, core_ids=list(range(NCORES)), trace=trace
    )
    if trace and res.exec_time_ns is not None:
        print(f"HW exec time: {res.exec_time_ns} ns")
        kernel.last_exec_time_ns = res.exec_time_ns
        kernel.last_trace = res.instructions_and_trace

    nblk = t_total // TBLK
    outs = []
    for r in res.results:
        a = r["hs"].reshape(nblk, 128, KC, TBLK, BL)
        # [blk, p, c, tp, b] -> [blk, tp, b, c, p] -> [T, BL, H]
        outs.append(
            np.ascontiguousarray(a.transpose(0, 3, 4, 2, 1)).reshape(t_total, BL, H)
        )
    return np.concatenate(outs, axis=1)


# revision 23
# speedup vs baseline: 1.0292x; 1.0292x over previous
"""LGRU Bass/Tile kernel for Trainium2, 8-core data-parallel over batch.

Reference computation (per sequence step t):
    xz = x @ Wz ; xh = x @ Wh                     (input projections)
    z  = sigmoid(xz_t + h @ Uz)
    hc = relu(xh_t + h @ Uh)
    h  = z * h + (1 - z) * hc
Returns all hidden states hs[T, B, H].

Sharding: batch (B=32) split 4-per-core across 8 cores; weights replicated.

Kernel design (v3):
  - h lives TRANSPOSED in SBUF as hsT[128, kc, t'*BL+b] (H on partitions)
    in bf16; the state buffer doubles as the block's output staging.
  - Per-step matmuls use U chunks as stationary bf16 operands with the
    tiny h slice moving, accumulating into per-step PSUM tiles that are
    PRE-FILLED with the x-projections via a bf16 identity matmul. Every
    per-step PSUM tile occupies a full 2 KiB bank (no false bank-sharing
    between the PE and the DVE/ACT readers).
  - The sigmoid runs ON THE VECTOR ENGINE via the Schraudolph exp bit
    trick (ScalarE's issue->semaphore-visible latency of ~1.1us would
    otherwise sit on the loop-carried path):
        m = A*az + B  (converted to int32; A=2^23/ln2, B=127*2^23-C)
        e = max(bitcast_f32(m), 0)    (clamp handles az < -8.8 garbage)
        w = 1/(1+e)  via reciprocal_approx_fast (51 ULP)  = 1 - z
        f = relu(ah) - h_prev         (fused scalar_tensor_tensor)
        h = h_prev + w * f
    f is written into d's buffer: the WAR hazard pins f behind the
    sigmoid chain in the DVE queue (the scheduler's cost model under-
    estimates the matmul bursts and would otherwise stall the DVE).
    End-to-end rel-L2 vs the fp32 reference: 6.4e-3 (gate is 2e-2).
  - Software pipelining: the loop body covers TWO blocks with ping-pong
    x-tile sets; block j+1's x DMA/transposes/projections are emitted
    interleaved into block j's recurrence steps. The projection PSUM
    tiles come from the SAME rotating PSUM ring as the per-step tiles,
    so ring reuse (write-after-read) pins each projection matmul near
    its emission slot — without that, the ASAP scheduler would front-
    load all prep at the block boundary and recreate the serial bubble.
    All prefetch copies (PSUM evacuation, casts) run on the otherwise
    idle Scalar engine. x is padded by one block host-side so the last
    prefetch never reads out of bounds.
  - Output is DMA'd in the transposed layout and un-transposed on the
    host (a PE-transpose + copy per block was pure overhead).
"""

import os

import numpy as np

T, B, F, H = 2048, 32, 256, 512
NCORES = 8
BL = B // NCORES  # batch per core = 4
TBLK = 128  # timesteps per block
KC = H // 128  # 4 H-chunks
FC = F // 128  # 2 F-chunks
PT = (TBLK * BL) // 128  # 4 partition-tiles of (t,b) rows per block (== KC)

SIG_A = float(2**23 / np.log(2))
SIG_B = float(127 * 2**23 - 500000)

_CACHED = {}


def _build_nc(t_total):
    import concourse.mybir as mybir
    from concourse import bacc
    import concourse.tile as tile
    from concourse.bass import ds
    from concourse.masks import make_identity

    FD = mybir.dt.float32
    BF = mybir.dt.bfloat16
    I32 = mybir.dt.int32
    nblk = t_total // TBLK
    paired = nblk % 2 == 0 and nblk >= 2

    nc = bacc.Bacc("TRN2", target_bir_lowering=False, debug=False)
    # one pad block so next-block prefetch never reads out of bounds
    x = nc.dram_tensor("x", [t_total + TBLK, BL, F], FD, kind="ExternalInput")
    Wz = nc.dram_tensor("Wz", [F, H], FD, kind="ExternalInput")
    Wh = nc.dram_tensor("Wh", [F, H], FD, kind="ExternalInput")
    Uz = nc.dram_tensor("Uz", [H, H], FD, kind="ExternalInput")
    Uh = nc.dram_tensor("Uh", [H, H], FD, kind="ExternalInput")
    # transposed output: hs[blk, p, c, tp*BL+b] = h[blk*TBLK+tp, b, c*128+p]
    hs = nc.dram_tensor(
        "hs", [nblk, 128, KC, TBLK * BL], FD, kind="ExternalOutput"
    )

    x_flat = x.rearrange("t b f -> (t b) f")
    hs_flat = hs.rearrange("a p c t -> (a p) c t")

    Alu = mybir.AluOpType

    with tile.TileContext(nc) as tc:
        with (
            tc.tile_pool(name="const", bufs=1) as constp,
            tc.tile_pool(name="setup", bufs=2) as setupp,
            tc.tile_pool(name="state", bufs=1) as statep,
            tc.tile_pool(name="xblk", bufs=1) as xblkp,
            tc.tile_pool(name="work", bufs=3) as workp,
            tc.tile_pool(name="step", bufs=3) as stepp,
            tc.tile_pool(name="ps_rec", bufs=6, space="PSUM") as ps_rec,
            tc.tile_pool(name="ps_tr", bufs=2, space="PSUM") as ps_tr,
        ):
            ident = constp.tile([128, 128], FD, tag="ident")
            make_identity(nc, ident)
            ident_b = constp.tile([128, 128], BF, tag="identb")
            nc.vector.tensor_copy(ident_b, ident)
            ones = constp.tile([128, KC, BL], FD, tag="ones")
            nc.vector.memset(ones, 1.0)

            # --- U blocks, single bf16 ---
            Ub = {}
            for g, Usrc in (("z", Uz), ("h", Uh)):
                for kc in range(KC):
                    stage = setupp.tile(
                        [128, H], FD, tag=f"stage{g}{kc}", name=f"stage{g}{kc}"
                    )
                    nc.sync.dma_start(out=stage, in_=Usrc[kc * 128 : (kc + 1) * 128, :])
                    ub = constp.tile([128, H], BF, tag=f"U{g}{kc}")
                    nc.vector.tensor_copy(ub, stage)
                    Ub[(g, kc)] = ub

            # --- W blocks, bf16: Wcat = [Wz | Wh] along output dim ---
            Wb = []
            for kc in range(FC):
                wtile = constp.tile([128, 2 * H], BF, tag=f"W{kc}")
                for si, Wsrc in enumerate((Wz, Wh)):
                    stage = setupp.tile(
                        [128, H], FD, tag=f"stageW{kc}{si}", name=f"stageW{kc}{si}"
                    )
                    nc.sync.dma_start(out=stage, in_=Wsrc[kc * 128 : (kc + 1) * 128, :])
                    nc.vector.tensor_copy(wtile[:, si * H : (si + 1) * H], stage)
                Wb.append(wtile)

            # --- persistent state: transposed h states for one block, bf16 ---
            hsT = statep.tile([128, KC, TBLK * BL], BF)
            nc.vector.memset(hsT[:, :, (TBLK - 1) * BL :], 0.0)

            # --- ping-pong x-tile sets ---
            def make_set(sfx):
                return {
                    "xT": [
                        xblkp.tile(
                            [128, KC, 128], BF, tag=f"xT{fc}{sfx}", name=f"xT{fc}{sfx}"
                        )
                        for fc in range(FC)
                    ],
                    "xzT": xblkp.tile(
                        [128, KC, KC, 128], BF, tag=f"xzT{sfx}", name=f"xzT{sfx}"
                    ),
                    "xhT": xblkp.tile(
                        [128, KC, KC, 128], BF, tag=f"xhT{sfx}", name=f"xhT{sfx}"
                    ),
                }

            set_a = make_set("a")
            set_b = make_set("b")

            def prep_items(dst, row0):
                """Closures loading/transforming block data at rows row0 into
                tile set dst. PE work rides the rec-PSUM ring (pinned); all
                copies run on the Scalar engine."""
                items = []
                shared = {}

                def mk_load(pt):
                    def it():
                        xin = workp.tile([128, F], FD, tag="xin", bufs=4, name="xin")
                        nc.sync.dma_start(
                            out=xin, in_=x_flat[ds(row0 + pt * 128, 128), :]
                        )
                        xb = workp.tile([128, F], BF, tag="xb", bufs=4, name="xb")
                        nc.scalar.copy(xb, xin)
                        shared[pt] = xb

                    return it

                def mk_tr(pt, fc):
                    def it():
                        pst = ps_tr.tile([128, 128], BF, tag="trb", name="pst")
                        nc.tensor.transpose(
                            pst, shared[pt][:, fc * 128 : (fc + 1) * 128], ident_b
                        )
                        nc.scalar.copy(dst["xT"][fc][:, pt, :], pst)

                    return it

                def mk_proj(mt):
                    def it():
                        psp = ps_rec.tile(
                            [128, KC, 128], FD, tag="rec", name="psp"
                        )
                        lhs_sl = slice(mt * 128, (mt + 1) * 128)
                        for kc in range(FC):
                            nc.tensor.matmul(
                                psp,
                                lhsT=Wb[kc][:, lhs_sl],
                                rhs=dst["xT"][kc],
                                start=(kc == 0),
                                stop=(kc == FC - 1),
                            )
                        d = dst["xzT"] if mt < KC else dst["xhT"]
                        nc.scalar.copy(d[:, mt % KC, :, :], psp)

                    return it

                for pt in range(PT):
                    items.append(mk_load(pt))
                    for fc in range(FC):
                        items.append(mk_tr(pt, fc))
                for mt in range(2 * KC):
                    items.append(mk_proj(mt))
                return items

            def emit_step(cur, tp, extra):
                q, r = divmod(tp * BL, 128)
                curs = ds(tp * BL, BL)
                prev = ds((tp - 1) * BL, BL) if tp > 0 else ds((TBLK - 1) * BL, BL)
                ps_z = ps_rec.tile([128, KC, 128], FD, tag="rec", name="ps_z")
                ps_h = ps_rec.tile([128, KC, 128], FD, tag="rec", name="ps_h")
                # prefill PSUM with the x-projections (identity matmul sets
                # has_written so the U matmuls accumulate on top)
                nc.tensor.matmul(
                    ps_z[:, :, 0:BL],
                    lhsT=ident_b,
                    rhs=cur["xzT"][:, :, q, r : r + BL],
                    start=True,
                    stop=False,
                )
                nc.tensor.matmul(
                    ps_h[:, :, 0:BL],
                    lhsT=ident_b,
                    rhs=cur["xhT"][:, :, q, r : r + BL],
                    start=True,
                    stop=False,
                )

                def u_burst(g, ps):
                    for mt in range(KC):
                        for kc in range(KC):
                            nc.tensor.matmul(
                                ps[:, mt, 0:BL],
                                lhsT=Ub[(g, kc)][:, mt * 128 : (mt + 1) * 128],
                                rhs=hsT[:, kc, prev],
                                start=False,
                                stop=(kc == KC - 1),
                                skip_group_check=True,
                            )

                u_burst("z", ps_z)
                # w = 1-z = 1/(1+exp(az)) via DVE exp bit trick, overlapping
                # the h matmul burst
                u_t = stepp.tile([128, KC, BL], I32, tag="u", name="u_t")
                nc.vector.tensor_scalar(
                    u_t, ps_z[:, :, 0:BL], SIG_A, SIG_B, Alu.mult, Alu.add
                )
                d_t = stepp.tile([128, KC, BL], FD, tag="d", name="d_t")
                nc.vector.tensor_scalar(
                    d_t, u_t.bitcast(FD), 0.0, 1.0, Alu.max, Alu.add
                )
                w_t = stepp.tile([128, KC, BL], FD, tag="w", name="w_t")
                # d is in [1, ~6600] — safely inside approx_fast's domain
                nc.vector.reciprocal_approx_fast(out=w_t, in_=d_t)
                u_burst("h", ps_h)
                # f = relu(ah) - h_prev, fused; written INTO d_t: the WAR
                # hazard (w reads d_t) pins f behind the sigmoid chain in
                # the DVE queue at zero cost
                f_t = d_t
                nc.vector.scalar_tensor_tensor(
                    f_t, ps_h[:, :, 0:BL], 0.0, hsT[:, :, prev], Alu.max, Alu.subtract
                )
                # h = h_prev + w*f, written straight into bf16 state
                g_t = stepp.tile([128, KC, BL], FD, tag="g", name="g_t")
                nc.vector.tensor_mul(g_t, w_t, f_t)
                nc.vector.tensor_add(hsT[:, :, curs], g_t, hsT[:, :, prev])
                if extra is not None:
                    extra(tp)

            def emit_block(cur, hsrow0, prefetch_items):
                if prefetch_items:
                    sched = {}
                    for k, it in enumerate(prefetch_items):
                        sched[2 + 3 * k] = it

                    def extra(tp):
                        if tp in sched:
                            sched[tp]()

                else:
                    extra = None
                for tp in range(TBLK):
                    emit_step(cur, tp, extra)
                # cast block states to f32 on the Scalar engine and DMA out
                hsF = workp.tile(
                    [128, KC, TBLK * BL], FD, tag="hsF", bufs=2, name="hsF"
                )
                for c in range(KC):
                    nc.scalar.copy(hsF[:, c, :], hsT[:, c, :])
                nc.sync.dma_start(out=hs_flat[ds(hsrow0, 128), :, :], in_=hsF)

            # prologue: block 0 into set A
            for it in prep_items(set_a, 0):
                it()

            if paired:
                with tc.For_i(0, nblk // 2, 1, staggered_reset=True) as pair:
                    row_even = pair * (2 * TBLK * BL)
                    emit_block(
                        set_a,
                        pair * 256,
                        prep_items(set_b, row_even + TBLK * BL),
                    )
                    emit_block(
                        set_b,
                        pair * 256 + 128,
                        prep_items(set_a, row_even + 2 * TBLK * BL),
                    )
            else:
                with tc.For_i(0, nblk, 1, staggered_reset=True) as blk:
                    emit_block(
                        set_a,
                        blk * 128,
                        prep_items(set_a, blk * (TBLK * BL) + TBLK * BL)
                        if nblk > 1
                        else None,
                    )

    nc.finalize()
    return nc


def kernel(x, Wz, Wh, Uz, Uh):
    from concourse.bass_utils import run_bass_kernel_spmd

    t_total = x.shape[0]
    if t_total not in _CACHED:
        _CACHED[t_total] = _build_nc(t_total)
    nc = _CACHED[t_total]

    x = np.asarray(x, dtype=np.float32)
    xpad = np.zeros((t_total + TBLK, x.shape[1], x.shape[2]), dtype=np.float32)
    xpad[:t_total] = x
    Wz = np.ascontiguousarray(np.asarray(Wz, dtype=np.float32))
    Wh = np.ascontiguousarray(np.asarray(Wh, dtype=np.float32))
    Uz = np.ascontiguousarray(np.asarray(Uz, dtype=np.float32))
    Uh = np.ascontiguousarray(np.asarray(Uh, dtype=np.float32))

    in_maps = []
    for c in range(NCORES):
        in_maps.append(
            {
                "x": np.ascontiguousarray(xpad[:, c * BL : (c + 1) * BL, :]),
                "Wz": Wz,
                "Wh": Wh,
                "Uz": Uz,
                "Uh": Uh,
            }
        )

    trace = os.environ.get("LGRU_TRACE", "0") == "1"
    res = run_bass_kernel_spmd(
        nc, in_maps, core_ids=list(range(NCORES)), trace=trace
    )
    if trace and res.exec_time_ns is not None:
        print(f"HW exec time: {res.exec_time_ns} ns")
        kernel.last_exec_time_ns = res.exec_time_ns
        kernel.last_trace = res.instructions_and_trace

    nblk = t_total // TBLK
    outs = []
    for r in res.results:
        a = r["hs"].reshape(nblk, 128, KC, TBLK, BL)
        # [blk, p, c, tp, b] -> [blk, tp, b, c, p] -> [T, BL, H]
        outs.append(
            np.ascontiguousarray(a.transpose(0, 3, 4, 2, 1)).reshape(t_total, BL, H)
        )
    return np.concatenate(outs, axis=1)


# revision 24
# speedup vs baseline: 1.1275x; 1.0956x over previous
"""LGRU Bass/Tile kernel for Trainium2, 8-core data-parallel over batch.

Reference computation (per sequence step t):
    xz = x @ Wz ; xh = x @ Wh                     (input projections)
    z  = sigmoid(xz_t + h @ Uz)
    hc = relu(xh_t + h @ Uh)
    h  = z * h + (1 - z) * hc
Returns all hidden states hs[T, B, H].

Sharding: batch (B=32) split 4-per-core across 8 cores; weights replicated.

Kernel design (v3):
  - h lives TRANSPOSED in SBUF as hsT[128, kc, t'*BL+b] (H on partitions)
    in bf16; the state buffer doubles as the block's output staging.
  - Per-step matmuls use U chunks as stationary bf16 operands with the
    tiny h slice moving, accumulating into per-step PSUM tiles that are
    PRE-FILLED with the x-projections via a bf16 identity matmul. Every
    per-step PSUM tile occupies a full 2 KiB bank (no false bank-sharing
    between the PE and the DVE/ACT readers).
  - The sigmoid runs ON THE VECTOR ENGINE via the Schraudolph exp bit
    trick (ScalarE's issue->semaphore-visible latency of ~1.1us would
    otherwise sit on the loop-carried path):
        m = A*az + B  (converted to int32; A=2^23/ln2, B=127*2^23-C)
        e = max(bitcast_f32(m), 0)    (clamp handles az < -8.8 garbage)
        w = 1/(1+e)  via reciprocal_approx_fast (51 ULP)  = 1 - z
        f = relu(ah) - h_prev         (fused scalar_tensor_tensor)
        h = h_prev + w * f
    f is written into d's buffer: the WAR hazard pins f behind the
    sigmoid chain in the DVE queue (the scheduler's cost model under-
    estimates the matmul bursts and would otherwise stall the DVE).
    End-to-end rel-L2 vs the fp32 reference: 6.4e-3 (gate is 2e-2).
  - Software pipelining: the loop body covers TWO blocks with ping-pong
    x-tile sets; block j+1's x DMA/transposes/projections are emitted
    interleaved into block j's recurrence steps. The projection PSUM
    tiles come from the SAME rotating PSUM ring as the per-step tiles,
    so ring reuse (write-after-read) pins each projection matmul near
    its emission slot — without that, the ASAP scheduler would front-
    load all prep at the block boundary and recreate the serial bubble.
    All prefetch copies (PSUM evacuation, casts) run on the otherwise
    idle Scalar engine. x is padded by one block host-side so the last
    prefetch never reads out of bounds.
  - Output is DMA'd in the transposed layout and un-transposed on the
    host (a PE-transpose + copy per block was pure overhead).
"""

import os

import numpy as np

T, B, F, H = 2048, 32, 256, 512
NCORES = 8
BL = B // NCORES  # batch per core = 4
TBLK = 128  # timesteps per block
KC = H // 128  # 4 H-chunks
FC = F // 128  # 2 F-chunks
PT = (TBLK * BL) // 128  # 4 partition-tiles of (t,b) rows per block (== KC)

SIG_A = float(2**23 / np.log(2))
SIG_B = float(127 * 2**23 - 500000)

_CACHED = {}


def _build_nc(t_total):
    import concourse.mybir as mybir
    from concourse import bacc
    import concourse.tile as tile
    from concourse.bass import ds
    from concourse.masks import make_identity

    FD = mybir.dt.float32
    BF = mybir.dt.bfloat16
    I32 = mybir.dt.int32
    nblk = t_total // TBLK
    paired = nblk % 2 == 0 and nblk >= 2

    nc = bacc.Bacc("TRN2", target_bir_lowering=False, debug=False)
    # one pad block so next-block prefetch never reads out of bounds
    x = nc.dram_tensor("x", [t_total + TBLK, BL, F], FD, kind="ExternalInput")
    Wz = nc.dram_tensor("Wz", [F, H], FD, kind="ExternalInput")
    Wh = nc.dram_tensor("Wh", [F, H], FD, kind="ExternalInput")
    Uz = nc.dram_tensor("Uz", [H, H], FD, kind="ExternalInput")
    Uh = nc.dram_tensor("Uh", [H, H], FD, kind="ExternalInput")
    # transposed output: hs[blk, p, c, tp*BL+b] = h[blk*TBLK+tp, b, c*128+p]
    hs = nc.dram_tensor(
        "hs", [nblk, 128, KC, TBLK * BL], FD, kind="ExternalOutput"
    )

    x_flat = x.rearrange("t b f -> (t b) f")
    hs_flat = hs.rearrange("a p c t -> (a p) c t")

    Alu = mybir.AluOpType

    with tile.TileContext(nc) as tc:
        with (
            tc.tile_pool(name="const", bufs=1) as constp,
            tc.tile_pool(name="setup", bufs=2) as setupp,
            tc.tile_pool(name="state", bufs=1) as statep,
            tc.tile_pool(name="xblk", bufs=1) as xblkp,
            tc.tile_pool(name="work", bufs=3) as workp,
            tc.tile_pool(name="step", bufs=3) as stepp,
            tc.tile_pool(name="ps_rec", bufs=4, space="PSUM") as ps_rec,
            tc.tile_pool(name="ps_live", bufs=1, space="PSUM") as ps_live,
            tc.tile_pool(name="ps_tr", bufs=2, space="PSUM") as ps_tr,
        ):
            ident = constp.tile([128, 128], FD, tag="ident")
            make_identity(nc, ident)
            ident_b = constp.tile([128, 128], BF, tag="identb")
            nc.vector.tensor_copy(ident_b, ident)
            ones = constp.tile([128, KC, BL], FD, tag="ones")
            nc.vector.memset(ones, 1.0)

            # --- U blocks, single bf16 ---
            Ub = {}
            for g, Usrc in (("z", Uz), ("h", Uh)):
                for kc in range(KC):
                    stage = setupp.tile(
                        [128, H], FD, tag=f"stage{g}{kc}", name=f"stage{g}{kc}"
                    )
                    nc.sync.dma_start(out=stage, in_=Usrc[kc * 128 : (kc + 1) * 128, :])
                    ub = constp.tile([128, H], BF, tag=f"U{g}{kc}")
                    nc.vector.tensor_copy(ub, stage)
                    Ub[(g, kc)] = ub

            # --- W blocks, bf16: Wcat = [Wz | Wh] along output dim ---
            Wb = []
            for kc in range(FC):
                wtile = constp.tile([128, 2 * H], BF, tag=f"W{kc}")
                for si, Wsrc in enumerate((Wz, Wh)):
                    stage = setupp.tile(
                        [128, H], FD, tag=f"stageW{kc}{si}", name=f"stageW{kc}{si}"
                    )
                    nc.sync.dma_start(out=stage, in_=Wsrc[kc * 128 : (kc + 1) * 128, :])
                    nc.vector.tensor_copy(wtile[:, si * H : (si + 1) * H], stage)
                Wb.append(wtile)

            # --- persistent state: transposed h states for one block (f32) ---
            hsT = statep.tile([128, KC, TBLK * BL], FD)
            nc.vector.memset(hsT[:, :, (TBLK - 1) * BL :], 0.0)
            # g = w*f update in bf16: the matmul moving operand (az/ah are
            # accumulated in persistent PSUM banks: az += dxz + Uz@g)
            g_bf = statep.tile([128, KC, BL], BF, name="g_bf")
            nc.vector.memset(g_bf, 0.0)
            ident_n = constp.tile([128, 128], BF, tag="identn")
            nc.scalar.mul(ident_n, ident, -1.0)

            # --- ping-pong x-tile sets ---
            def make_set(sfx):
                return {
                    "xT": [
                        xblkp.tile(
                            [128, KC, 128], BF, tag=f"xT{fc}{sfx}", name=f"xT{fc}{sfx}"
                        )
                        for fc in range(FC)
                    ],
                    "xzT": xblkp.tile(
                        [128, KC, KC, 128], BF, tag=f"xzT{sfx}", name=f"xzT{sfx}"
                    ),
                    "xhT": xblkp.tile(
                        [128, KC, KC, 128], BF, tag=f"xhT{sfx}", name=f"xhT{sfx}"
                    ),
                }

            set_a = make_set("a")
            set_b = make_set("b")

            def prep_items(dst, row0):
                """Closures loading/transforming block data at rows row0 into
                tile set dst. PE work rides the rec-PSUM ring (pinned); all
                copies run on the Scalar engine."""
                items = []
                shared = {}

                def mk_load(pt):
                    def it():
                        xin = workp.tile([128, F], FD, tag="xin", bufs=4, name="xin")
                        nc.sync.dma_start(
                            out=xin, in_=x_flat[ds(row0 + pt * 128, 128), :]
                        )
                        xb = workp.tile([128, F], BF, tag="xb", bufs=4, name="xb")
                        nc.scalar.copy(xb, xin)
                        shared[pt] = xb

                    return it

                def mk_tr(pt, fc):
                    def it():
                        pst = ps_tr.tile([128, 128], BF, tag="trb", name="pst")
                        nc.tensor.transpose(
                            pst, shared[pt][:, fc * 128 : (fc + 1) * 128], ident_b
                        )
                        nc.scalar.copy(dst["xT"][fc][:, pt, :], pst)

                    return it

                def mk_proj(mt):
                    def it():
                        psp = ps_rec.tile(
                            [128, KC, 128], FD, tag="rec", name="psp"
                        )
                        lhs_sl = slice(mt * 128, (mt + 1) * 128)
                        for kc in range(FC):
                            nc.tensor.matmul(
                                psp,
                                lhsT=Wb[kc][:, lhs_sl],
                                rhs=dst["xT"][kc],
                                start=(kc == 0),
                                stop=(kc == FC - 1),
                            )
                        d = dst["xzT"] if mt < KC else dst["xhT"]
                        nc.scalar.copy(d[:, mt % KC, :, :], psp)

                    return it

                for pt in range(PT):
                    items.append(mk_load(pt))
                    for fc in range(FC):
                        items.append(mk_tr(pt, fc))
                for mt in range(2 * KC):
                    items.append(mk_proj(mt))
                return items

            def emit_step(cur, nxt, tp, extra):
                curs = ds(tp * BL, BL)
                prev = ds((tp - 1) * BL, BL) if tp > 0 else ds((TBLK - 1) * BL, BL)

                def u_burst(g, ps):
                    for mt in range(KC):
                        for kc in range(KC):
                            nc.tensor.matmul(
                                ps[:, mt, 0:BL],
                                lhsT=Ub[(g, kc)][:, mt * 128 : (mt + 1) * 128],
                                rhs=g_bf[:, kc, :],
                                start=False,
                                stop=(kc == KC - 1),
                                skip_group_check=True,
                            )

                u_burst("z", lz)
                # w = 1-z = 1/(1+exp(az)) via DVE exp bit trick, overlapping
                # the h matmul burst
                u_t = stepp.tile([128, KC, BL], I32, tag="u", name="u_t")
                nc.vector.tensor_scalar(
                    u_t, lz[:, :, 0:BL], SIG_A, SIG_B, Alu.mult, Alu.add
                )
                d_t = stepp.tile([128, KC, BL], FD, tag="d", name="d_t")
                nc.vector.tensor_scalar(
                    d_t, u_t.bitcast(FD), 0.0, 1.0, Alu.max, Alu.add
                )
                w_t = stepp.tile([128, KC, BL], FD, tag="w", name="w_t")
                # d is in [1, ~6600] — safely inside approx_fast's domain
                nc.vector.reciprocal_approx_fast(out=w_t, in_=d_t)
                u_burst("h", lh)
                # f = relu(ah) - h_prev, fused; written INTO d_t: the WAR
                # hazard (w reads d_t) pins f behind the sigmoid chain in
                # the DVE queue at zero cost
                f_t = d_t
                nc.vector.scalar_tensor_tensor(
                    f_t, lh[:, :, 0:BL], 0.0, hsT[:, :, prev], Alu.max, Alu.subtract
                )
                # g = w*f (bf16, feeds the next step's bursts); the h update
                # is OFF the loop-carried ring (the bursts consume g, not h)
                nc.vector.tensor_mul(g_bf, w_t, f_t)
                nc.vector.tensor_add(hsT[:, :, curs], g_bf, hsT[:, :, prev])
                # advance the PSUM accumulators to step tp+1:
                #   az += xz[tp+1] - xz[tp]   (identity / neg-identity MMs)
                q, r = divmod(tp * BL, 128)
                if tp < TBLK - 1:
                    nq, nr = divmod((tp + 1) * BL, 128)
                    nxz, nxh = cur["xzT"], cur["xhT"]
                else:
                    nq, nr = 0, 0
                    nxz = nxt["xzT"] if nxt is not None else None
                    nxh = nxt["xhT"] if nxt is not None else None
                if nxz is not None:
                    for ps, xcur, xnxt in ((lz, cur["xzT"], nxz), (lh, cur["xhT"], nxh)):
                        nc.tensor.matmul(
                            ps[:, :, 0:BL],
                            lhsT=ident_b,
                            rhs=xnxt[:, :, nq, nr : nr + BL],
                            start=False,
                            stop=False,
                            skip_group_check=True,
                        )
                        nc.tensor.matmul(
                            ps[:, :, 0:BL],
                            lhsT=ident_n,
                            rhs=xcur[:, :, q, r : r + BL],
                            start=False,
                            stop=False,
                            skip_group_check=True,
                        )
                if extra is not None:
                    extra(tp)

            def emit_block(cur, nxt, hsrow0, prefetch_items):
                if prefetch_items:
                    sched = {}
                    for k, it in enumerate(prefetch_items):
                        sched[2 + 3 * k] = it

                    def extra(tp):
                        if tp in sched:
                            sched[tp]()

                else:
                    extra = None
                for tp in range(TBLK):
                    emit_step(cur, nxt, tp, extra)
                # cast block states to f32 on the Scalar engine and DMA out
                hsF = workp.tile(
                    [128, KC, TBLK * BL], FD, tag="hsF", bufs=2, name="hsF"
                )
                for c in range(KC):
                    nc.scalar.copy(hsF[:, c, :], hsT[:, c, :])
                nc.sync.dma_start(out=hs_flat[ds(hsrow0, 128), :, :], in_=hsF)

            # prologue: block 0 into set A, then init az/ah = xz/xh[0]
            for it in prep_items(set_a, 0):
                it()
            lz = ps_live.tile([128, KC, 128], FD, tag="lz", name="lz")
            lh = ps_live.tile([128, KC, 128], FD, tag="lh", name="lh")
            nc.tensor.matmul(
                lz[:, :, 0:BL], lhsT=ident_b, rhs=set_a["xzT"][:, :, 0, 0:BL],
                start=True, stop=False,
            )
            nc.tensor.matmul(
                lh[:, :, 0:BL], lhsT=ident_b, rhs=set_a["xhT"][:, :, 0, 0:BL],
                start=True, stop=False,
            )

            if paired:
                with tc.For_i(0, nblk // 2, 1, staggered_reset=True) as pair:
                    row_even = pair * (2 * TBLK * BL)
                    emit_block(
                        set_a,
                        set_b,
                        pair * 256,
                        prep_items(set_b, row_even + TBLK * BL),
                    )
                    emit_block(
                        set_b,
                        set_a,
                        pair * 256 + 128,
                        prep_items(set_a, row_even + 2 * TBLK * BL),
                    )
            else:
                with tc.For_i(0, nblk, 1, staggered_reset=True) as blk:
                    emit_block(
                        set_a,
                        None,
                        blk * 128,
                        prep_items(set_a, blk * (TBLK * BL) + TBLK * BL)
                        if nblk > 1
                        else None,
                    )

    nc.finalize()
    return nc


def kernel(x, Wz, Wh, Uz, Uh):
    from concourse.bass_utils import run_bass_kernel_spmd

    t_total = x.shape[0]
    if t_total not in _CACHED:
        _CACHED[t_total] = _build_nc(t_total)
    nc = _CACHED[t_total]

    x = np.asarray(x, dtype=np.float32)
    xpad = np.zeros((t_total + TBLK, x.shape[1], x.shape[2]), dtype=np.float32)
    xpad[:t_total] = x
    Wz = np.ascontiguousarray(np.asarray(Wz, dtype=np.float32))
    Wh = np.ascontiguousarray(np.asarray(Wh, dtype=np.float32))
    Uz = np.ascontiguousarray(np.asarray(Uz, dtype=np.float32))
    Uh = np.ascontiguousarray(np.asarray(Uh, dtype=np.float32))

    in_maps = []
    for c in range(NCORES):
        in_maps.append(
            {
                "x": np.ascontiguousarray(xpad[:, c * BL : (c + 1) * BL, :]),
                "Wz": Wz,
                "Wh": Wh,
                "Uz": Uz,
                "Uh": Uh,
            }
        )

    trace = os.environ.get("LGRU_TRACE", "0") == "1"
    res = run_bass_kernel_spmd(
        nc, in_maps, core_ids=list(range(NCORES)), trace=trace
    )
    if trace and res.exec_time_ns is not None:
        print(f"HW exec time: {res.exec_time_ns} ns")
        kernel.last_exec_time_ns = res.exec_time_ns
        kernel.last_trace = res.instructions_and_trace

    nblk = t_total // TBLK
    outs = []
    for r in res.results:
        a = r["hs"].reshape(nblk, 128, KC, TBLK, BL)
        # [blk, p, c, tp, b] -> [blk, tp, b, c, p] -> [T, BL, H]
        outs.append(
            np.ascontiguousarray(a.transpose(0, 3, 4, 2, 1)).reshape(t_total, BL, H)
        )
    return np.concatenate(outs, axis=1)
